# revision 8
# baseline (speedup 1.0000x reference)
"""Involution layer (per-pixel dynamic 3x3 grouped filtering) on 8 trn2 cores.

Sharding: data-parallel over (batch, h-block): core i owns batch i//2,
h rows [(i%2)*64, (i%2)*64+64). Each core gets a channel-major x slab
x_T[chunk, c, row, w+halo] with one halo row each side and zero halo
columns, built on the host, so the device kernel is purely local with
no on-chip transposes.

Per-core pipeline (fp32), processed in 4-row blocks (n = 4*128 = 512 pixels):
  - DMA x_T window [c, 6, 130] per chunk straight into SBUF
  - h = W1^T x (PE, K=c contracted in 2 chunks of 128) -> PSUM
  - hrelu = Relu(scale*h + bias) on ACT (BN affine folded in); ones row via GPSIMD
  - kerT[n,144] = hrelu_aug^T @ w2aug (PE, bias via ones row) -> kernel output
  - kerb_k[128,n] = w2xk_k^T @ hrelu_aug (PE): group-broadcast kernels,
    w2xk_k[d, m] = w2aug[d, k*16 + m%16]
  - involution products P_jk = xT_shift_k * kerb_k on DVE (9 mults/chunk)
  - sum over k on PE: out_j += I^T @ P_jk (accumulating identity matmuls)
  - ACT copies sums PSUM->SBUF; DMA to channel-major out_T; host transposes back.
"""
import os
import sys
import numpy as np

sys.path.insert(0, "/opt/trn_rl_repo")

import concourse.bass as bass
import concourse.bacc as bacc
import concourse.tile as tile
from concourse import mybir
from concourse.bass_utils import run_bass_kernel_spmd
from concourse.masks import make_identity

BN_EPS = 1e-3
B, H, W, C = 4, 128, 128, 256
G, K, RED = 16, 3, 4
K2 = K * K
CR = C // RED          # 64
E = K2 * G             # 144
ROWS = 64              # output rows per core
SLAB = ROWS + 2        # input rows incl halo
R = 4                  # rows per block
NBLK = ROWS // R
N = R * W              # 512 pixels per block
WH = W + 2             # w incl halo cols
F32 = mybir.dt.float32

_CACHE = {}


def _build_program():
    nc = bacc.Bacc("TRN2", target_bir_lowering=False, debug=False, num_devices=8)

    x_dram = nc.dram_tensor("x_t", [2, 128, SLAB, WH], F32, kind="ExternalInput").ap()
    w1_dram = nc.dram_tensor("w1c", [2, 128, CR], F32, kind="ExternalInput").ap()
    scale_dram = nc.dram_tensor("actscale", [CR, 1], F32, kind="ExternalInput").ap()
    bias_dram = nc.dram_tensor("actbias", [CR, 1], F32, kind="ExternalInput").ap()
    w2aug_dram = nc.dram_tensor("w2aug", [CR + 1, E], F32, kind="ExternalInput").ap()
    w2xk_dram = nc.dram_tensor("w2xk", [K2, CR + 1, 128], F32, kind="ExternalInput").ap()
    out_dram = nc.dram_tensor("out_t", [2, 128, ROWS, W], F32, kind="ExternalOutput").ap()
    ker_dram = nc.dram_tensor("ker_slab", [ROWS, W, E], F32, kind="ExternalOutput").ap()

    with tile.TileContext(nc) as tc:
        _kernel_body(tc, x_dram, w1_dram, scale_dram, bias_dram, w2aug_dram,
                     w2xk_dram, out_dram, ker_dram)
    nc.compile()
    return nc


def _kernel_body(tc, x_dram, w1_dram, scale_dram, bias_dram, w2aug_dram,
                 w2xk_dram, out_dram, ker_dram):
    nc = tc.nc
    from contextlib import ExitStack
    ctx = ExitStack()
    with ctx:
        consts = ctx.enter_context(tc.tile_pool(name="consts", bufs=1))
        xt_p = ctx.enter_context(tc.tile_pool(name="xt", bufs=2))
        hr_p = ctx.enter_context(tc.tile_pool(name="hr", bufs=2))
        stage_p = ctx.enter_context(tc.tile_pool(name="stage", bufs=3))
        prod_p = ctx.enter_context(tc.tile_pool(name="prod", bufs=20))
        ps_hk = ctx.enter_context(tc.tile_pool(name="ps_hk", bufs=2, space="PSUM"))
        ps_kb = ctx.enter_context(tc.tile_pool(name="ps_kb", bufs=3, space="PSUM"))
        ps_sum = ctx.enter_context(tc.tile_pool(name="ps_sum", bufs=1, space="PSUM"))

        # ---- constants ----
        w1_sb = consts.tile([128, 2, CR], F32)
        nc.sync.dma_start(out=w1_sb, in_=w1_dram.rearrange("j p d -> p j d"))
        scale_sb = consts.tile([CR, 1], F32)
        nc.sync.dma_start(out=scale_sb, in_=scale_dram)
        bias_sb = consts.tile([CR, 1], F32)
        nc.sync.dma_start(out=bias_sb, in_=bias_dram)
        w2aug_sb = consts.tile([CR + 1, E], F32)
        nc.sync.dma_start(out=w2aug_sb, in_=w2aug_dram)
        w2xk_sb = consts.tile([CR + 1, K2, 128], F32)
        nc.sync.dma_start(out=w2xk_sb, in_=w2xk_dram.rearrange("k d m -> d k m"))
        ident = consts.tile([128, 128], F32)
        make_identity(nc, ident)

        for blk in range(NBLK):
            r0 = R * blk

            # ---- load channel-major x window [128, chunk, 6, 130] ----
            xT = xt_p.tile([128, 2, R + 2, WH], F32, tag="xT")
            for j in range(2):
                nc.sync.dma_start(out=xT[:, j], in_=x_dram[j, :, r0:r0 + R + 2, :])

            # ---- kernel generation ----
            h_ps = ps_hk.tile([CR, N], F32, tag="hk")
            for j in range(2):
                nc.tensor.matmul(h_ps, lhsT=w1_sb[:, j, :],
                                 rhs=xT[:, j, 1:R + 1, 1:W + 1],
                                 start=(j == 0), stop=(j == 1))
            hrelu_aug = hr_p.tile([CR + 1, N], F32, tag="hrelu")
            nc.gpsimd.memset(hrelu_aug[CR:CR + 1, :], 1.0)
            nc.scalar.activation(hrelu_aug[0:CR, :], h_ps,
                                 mybir.ActivationFunctionType.Relu,
                                 bias=bias_sb, scale=scale_sb)

            ksb = stage_p.tile([128, R, E], F32, tag="ksb")
            for half in range(2):
                kt = ps_hk.tile([128, 2, E], F32, tag="hk")
                for i2 in range(2):
                    i = half * 2 + i2
                    nc.tensor.matmul(kt[:, i2, :],
                                     lhsT=hrelu_aug[:, i * 128:(i + 1) * 128],
                                     rhs=w2aug_sb, start=True, stop=True)
                nc.scalar.copy(ksb[:, half * 2:half * 2 + 2, :], kt)
            nc.sync.dma_start(out=ker_dram[r0:r0 + R].rearrange("r w e -> w r e"),
                              in_=ksb)

            # ---- group-broadcast kernels + involution ----
            sums = []
            for j in range(2):
                sums.append(ps_sum.tile([128, R, W], F32, tag=f"sum{j}",
                                        name=f"sum{j}"))
            prods = []
            for k in range(K2):
                di, dj = k // 3, k % 3
                kb = ps_kb.tile([128, R, W], F32, tag="kerb")
                nc.tensor.matmul(kb, lhsT=w2xk_sb[:, k, :], rhs=hrelu_aug,
                                 start=True, stop=True)
                for j in range(2):
                    prod = prod_p.tile([128, R, W], F32, tag="prod",
                                       name=f"prod{k}_{j}")
                    nc.vector.tensor_mul(prod, xT[:, j, di:di + R, dj:dj + W], kb)
                    prods.append(prod)
            _skip_sums = os.environ.get("KSKIP_SUMS") == "1"
            for k in range(1 if _skip_sums else K2):
                for j in range(2):
                    nc.tensor.matmul(sums[j], lhsT=ident, rhs=prods[2 * k + j],
                                     start=(k == 0),
                                     stop=(k == (0 if _skip_sums else K2 - 1)),
                                     skip_group_check=True)

            # ---- copy out and store (channel-major; host transposes back) ----
            for j in range(2):
                osb = stage_p.tile([128, R, W], F32, tag="osb")
                nc.scalar.copy(osb, sums[j])
                nc.sync.dma_start(out=out_dram[j, :, r0:r0 + R, :], in_=osb)


def _host_prep(w1, b1, gamma, beta, bn_mean, bn_var, w2, b2):
    s = (gamma / np.sqrt(bn_var + BN_EPS)).astype(np.float32)
    actscale = np.ascontiguousarray(s[:, None])
    actbias = np.ascontiguousarray((((b1 - bn_mean) * s) + beta)[:, None]).astype(np.float32)
    w1c = np.ascontiguousarray(w1.reshape(2, 128, CR)).astype(np.float32)
    w2aug = np.vstack([w2, b2[None]]).astype(np.float32)
    m = np.arange(128)
    w2xk = np.ascontiguousarray(
        np.stack([w2aug[:, k * G + (m % G)] for k in range(K2)])).astype(np.float32)
    return actscale, actbias, w1c, w2aug, w2xk


def kernel(x, w1, b1, gamma, beta, bn_mean, bn_var, w2, b2, _profile=None):
    x = np.asarray(x, np.float32)
    actscale, actbias, w1c, w2aug, w2xk = _host_prep(
        np.asarray(w1, np.float32), np.asarray(b1, np.float32),
        np.asarray(gamma, np.float32), np.asarray(beta, np.float32),
        np.asarray(bn_mean, np.float32), np.asarray(bn_var, np.float32),
        np.asarray(w2, np.float32), np.asarray(b2, np.float32))

    if "nc" not in _CACHE:
        _CACHE["nc"] = _build_program()
    nc = _CACHE["nc"]

    in_maps = []
    for i in range(8):
        bi, h0 = i // 2, (i % 2) * ROWS
        xs = np.zeros((SLAB, W, C), np.float32)
        xs[1:1 + ROWS] = x[bi, h0:h0 + ROWS]
        if h0 > 0:
            xs[0] = x[bi, h0 - 1]
        if h0 + ROWS < H:
            xs[1 + ROWS] = x[bi, h0 + ROWS]
        # channel-major with zero halo cols: [2, 128, SLAB, W+2]
        xt = np.zeros((2, 128, SLAB, WH), np.float32)
        xt[:, :, :, 1:W + 1] = (
            xs.transpose(2, 0, 1).reshape(2, 128, SLAB, W))
        in_maps.append({
            "x_t": xt, "w1c": w1c, "actscale": actscale, "actbias": actbias,
            "w2aug": w2aug, "w2xk": w2xk,
        })

    kwargs = dict(_profile) if _profile else {}
    res = run_bass_kernel_spmd(nc, in_maps, list(range(8)), **kwargs)
    if _profile is not None:
        _CACHE["last_result"] = res

    out = np.empty((B, H, W, C), np.float32)
    ker = np.empty((B, H, W, E), np.float32)
    for i in range(8):
        bi, h0 = i // 2, (i % 2) * ROWS
        # out_t [2, 128, ROWS, W] -> [ROWS, W, 2*128]
        ot = res.results[i]["out_t"].reshape(C, ROWS, W)
        out[bi, h0:h0 + ROWS] = ot.transpose(1, 2, 0)
        ker[bi, h0:h0 + ROWS] = res.results[i]["ker_slab"]
    return out, ker.reshape(B, H, W, K2, 1, G)


# revision 9
# speedup vs baseline: 65808.0371x; 65808.0371x over previous
"""Involution layer (per-pixel dynamic 3x3 grouped filtering) on 8 trn2 cores.

Sharding: data-parallel over (batch, h-block): core i owns batch i//2,
h rows [(i%2)*64, (i%2)*64+64). Each core gets a channel-major x slab
x_T[chunk, c, row, w+halo] with one halo row each side and zero halo
columns, built on the host, so the device kernel is purely local with
no on-chip transposes.

Per-core pipeline (fp32), processed in 4-row blocks (n = 4*128 = 512 pixels):
  - DMA x_T window [c, 6, 130] per chunk straight into SBUF
  - h = W1^T x (PE, K=c contracted in 2 chunks of 128) -> PSUM
  - hrelu = Relu(scale*h + bias) on ACT (BN affine folded in); ones row via GPSIMD
  - kerT[n,144] = hrelu_aug^T @ w2aug (PE, bias via ones row) -> kernel output
  - kerb_k[128,n] = w2xk_k^T @ hrelu_aug (PE): group-broadcast kernels,
    w2xk_k[d, m] = w2aug[d, k*16 + m%16]
  - involution products P_jk = xT_shift_k * kerb_k on DVE (9 mults/chunk)
  - sum over k on PE: out_j += I^T @ P_jk (accumulating identity matmuls)
  - ACT copies sums PSUM->SBUF; DMA to channel-major out_T; host transposes back.
"""
import os
import sys
import numpy as np

sys.path.insert(0, "/opt/trn_rl_repo")

import concourse.bass as bass
import concourse.bacc as bacc
import concourse.tile as tile
from concourse import mybir
from concourse.bass_utils import run_bass_kernel_spmd
from concourse.masks import make_identity

BN_EPS = 1e-3
B, H, W, C = 4, 128, 128, 256
G, K, RED = 16, 3, 4
K2 = K * K
CR = C // RED          # 64
E = K2 * G             # 144
ROWS = 64              # output rows per core
SLAB = ROWS + 2        # input rows incl halo
R = 4                  # rows per block
NBLK = ROWS // R
N = R * W              # 512 pixels per block
WH = W + 2             # w incl halo cols
F32 = mybir.dt.float32

_CACHE = {}


def _build_program():
    nc = bacc.Bacc("TRN2", target_bir_lowering=False, debug=False, num_devices=8)

    x_dram = nc.dram_tensor("x_t", [2, 128, SLAB, WH], F32, kind="ExternalInput").ap()
    w1_dram = nc.dram_tensor("w1c", [2, 128, CR], F32, kind="ExternalInput").ap()
    scale_dram = nc.dram_tensor("actscale", [CR, 1], F32, kind="ExternalInput").ap()
    bias_dram = nc.dram_tensor("actbias", [CR, 1], F32, kind="ExternalInput").ap()
    w2aug_dram = nc.dram_tensor("w2aug", [CR + 1, E], F32, kind="ExternalInput").ap()
    w2xk_dram = nc.dram_tensor("w2xk", [K2, CR + 1, 128], F32, kind="ExternalInput").ap()
    out_dram = nc.dram_tensor("out_t", [2, 128, ROWS, W], F32, kind="ExternalOutput").ap()
    ker_dram = nc.dram_tensor("ker_slab", [ROWS, W, E], F32, kind="ExternalOutput").ap()

    with tile.TileContext(nc) as tc:
        _kernel_body(tc, x_dram, w1_dram, scale_dram, bias_dram, w2aug_dram,
                     w2xk_dram, out_dram, ker_dram)
    nc.compile()
    return nc


def _kernel_body(tc, x_dram, w1_dram, scale_dram, bias_dram, w2aug_dram,
                 w2xk_dram, out_dram, ker_dram):
    nc = tc.nc
    from contextlib import ExitStack
    ctx = ExitStack()
    with ctx:
        consts = ctx.enter_context(tc.tile_pool(name="consts", bufs=1))
        xt_p = ctx.enter_context(tc.tile_pool(name="xt", bufs=2))
        hr_p = ctx.enter_context(tc.tile_pool(name="hr", bufs=2))
        stage_p = ctx.enter_context(tc.tile_pool(name="stage", bufs=3))
        prod_p = ctx.enter_context(tc.tile_pool(name="prod", bufs=20))
        ps_hk = ctx.enter_context(tc.tile_pool(name="ps_hk", bufs=2, space="PSUM"))
        ps_kb = ctx.enter_context(tc.tile_pool(name="ps_kb", bufs=3, space="PSUM"))
        ps_sum = ctx.enter_context(tc.tile_pool(name="ps_sum", bufs=1, space="PSUM"))

        # ---- constants ----
        w1_sb = consts.tile([128, 2, CR], F32)
        nc.sync.dma_start(out=w1_sb, in_=w1_dram.rearrange("j p d -> p j d"))
        scale_sb = consts.tile([CR, 1], F32)
        nc.sync.dma_start(out=scale_sb, in_=scale_dram)
        bias_sb = consts.tile([CR, 1], F32)
        nc.sync.dma_start(out=bias_sb, in_=bias_dram)
        w2aug_sb = consts.tile([CR + 1, E], F32)
        nc.sync.dma_start(out=w2aug_sb, in_=w2aug_dram)
        w2xk_sb = consts.tile([CR + 1, K2, 128], F32)
        nc.sync.dma_start(out=w2xk_sb, in_=w2xk_dram.rearrange("k d m -> d k m"))
        ident = consts.tile([128, 128], F32)
        make_identity(nc, ident)

        for blk in range(NBLK):
            r0 = R * blk

            # ---- load channel-major x window [128, chunk, 6, 130] ----
            xT = xt_p.tile([128, 2, R + 2, WH], F32, tag="xT")
            for j in range(2):
                nc.sync.dma_start(out=xT[:, j], in_=x_dram[j, :, r0:r0 + R + 2, :])

            # ---- kernel generation ----
            h_ps = ps_hk.tile([CR, N], F32, tag="hk")
            for j in range(2):
                nc.tensor.matmul(h_ps, lhsT=w1_sb[:, j, :],
                                 rhs=xT[:, j, 1:R + 1, 1:W + 1],
                                 start=(j == 0), stop=(j == 1))
            hrelu_aug = hr_p.tile([CR + 1, N], F32, tag="hrelu")
            nc.gpsimd.memset(hrelu_aug[CR:CR + 1, :], 1.0)
            nc.scalar.activation(hrelu_aug[0:CR, :], h_ps,
                                 mybir.ActivationFunctionType.Relu,
                                 bias=bias_sb, scale=scale_sb)

            ksb = stage_p.tile([128, R, E], F32, tag="ksb")
            for half in range(2):
                kt = ps_hk.tile([128, 2, E], F32, tag="hk")
                for i2 in range(2):
                    i = half * 2 + i2
                    nc.tensor.matmul(kt[:, i2, :],
                                     lhsT=hrelu_aug[:, i * 128:(i + 1) * 128],
                                     rhs=w2aug_sb, start=True, stop=True)
                nc.scalar.copy(ksb[:, half * 2:half * 2 + 2, :], kt)
            nc.sync.dma_start(out=ker_dram[r0:r0 + R].rearrange("r w e -> w r e"),
                              in_=ksb)

            # ---- group-broadcast kernels + involution ----
            sums = []
            for j in range(2):
                sums.append(ps_sum.tile([128, R, W], F32, tag=f"sum{j}",
                                        name=f"sum{j}"))
            prods = []
            for k in range(K2):
                di, dj = k // 3, k % 3
                kb = ps_kb.tile([128, R, W], F32, tag="kerb")
                nc.tensor.matmul(kb, lhsT=w2xk_sb[:, k, :], rhs=hrelu_aug,
                                 start=True, stop=True)
                for j in range(2):
                    prod = prod_p.tile([128, R, W], F32, tag="prod",
                                       name=f"prod{k}_{j}")
                    nc.vector.tensor_mul(prod, xT[:, j, di:di + R, dj:dj + W], kb)
                    prods.append(prod)
            for k in range(K2):
                for j in range(2):
                    nc.tensor.matmul(sums[j], lhsT=ident, rhs=prods[2 * k + j],
                                     start=(k == 0), stop=(k == K2 - 1),
                                     skip_group_check=True)

            # ---- copy out and store (channel-major; host transposes back) ----
            for j in range(2):
                osb = stage_p.tile([128, R, W], F32, tag="osb")
                nc.scalar.copy(osb, sums[j])
                nc.sync.dma_start(out=out_dram[j, :, r0:r0 + R, :], in_=osb)


def _host_prep(w1, b1, gamma, beta, bn_mean, bn_var, w2, b2):
    s = (gamma / np.sqrt(bn_var + BN_EPS)).astype(np.float32)
    actscale = np.ascontiguousarray(s[:, None])
    actbias = np.ascontiguousarray((((b1 - bn_mean) * s) + beta)[:, None]).astype(np.float32)
    w1c = np.ascontiguousarray(w1.reshape(2, 128, CR)).astype(np.float32)
    w2aug = np.vstack([w2, b2[None]]).astype(np.float32)
    m = np.arange(128)
    w2xk = np.ascontiguousarray(
        np.stack([w2aug[:, k * G + (m % G)] for k in range(K2)])).astype(np.float32)
    return actscale, actbias, w1c, w2aug, w2xk


def kernel(x, w1, b1, gamma, beta, bn_mean, bn_var, w2, b2, _profile=None):
    x = np.asarray(x, np.float32)
    actscale, actbias, w1c, w2aug, w2xk = _host_prep(
        np.asarray(w1, np.float32), np.asarray(b1, np.float32),
        np.asarray(gamma, np.float32), np.asarray(beta, np.float32),
        np.asarray(bn_mean, np.float32), np.asarray(bn_var, np.float32),
        np.asarray(w2, np.float32), np.asarray(b2, np.float32))

    if "nc" not in _CACHE:
        _CACHE["nc"] = _build_program()
    nc = _CACHE["nc"]

    in_maps = []
    for i in range(8):
        bi, h0 = i // 2, (i % 2) * ROWS
        xs = np.zeros((SLAB, W, C), np.float32)
        xs[1:1 + ROWS] = x[bi, h0:h0 + ROWS]
        if h0 > 0:
            xs[0] = x[bi, h0 - 1]
        if h0 + ROWS < H:
            xs[1 + ROWS] = x[bi, h0 + ROWS]
        # channel-major with zero halo cols: [2, 128, SLAB, W+2]
        xt = np.zeros((2, 128, SLAB, WH), np.float32)
        xt[:, :, :, 1:W + 1] = (
            xs.transpose(2, 0, 1).reshape(2, 128, SLAB, W))
        in_maps.append({
            "x_t": xt, "w1c": w1c, "actscale": actscale, "actbias": actbias,
            "w2aug": w2aug, "w2xk": w2xk,
        })

    kwargs = dict(_profile) if _profile else {}
    res = run_bass_kernel_spmd(nc, in_maps, list(range(8)), **kwargs)
    if _profile is not None:
        _CACHE["last_result"] = res

    out = np.empty((B, H, W, C), np.float32)
    ker = np.empty((B, H, W, E), np.float32)
    for i in range(8):
        bi, h0 = i // 2, (i % 2) * ROWS
        # out_t [2, 128, ROWS, W] -> [ROWS, W, 2*128]
        ot = res.results[i]["out_t"].reshape(C, ROWS, W)
        out[bi, h0:h0 + ROWS] = ot.transpose(1, 2, 0)
        ker[bi, h0:h0 + ROWS] = res.results[i]["ker_slab"]
    return out, ker.reshape(B, H, W, K2, 1, G)


# revision 10
# speedup vs baseline: 65819.5936x; 1.0002x over previous
"""Involution layer (per-pixel dynamic 3x3 grouped filtering) on 8 trn2 cores.

Sharding: data-parallel over (batch, h-block): core i owns batch i//2,
h rows [(i%2)*64, (i%2)*64+64). Each core gets a channel-major x slab
x_T[chunk, c, row, w+halo] with one halo row each side and zero halo
columns, built on the host, so the device kernel is purely local with
no on-chip transposes.

Per-core pipeline (fp32), processed in 4-row blocks (n = 4*128 = 512 pixels):
  - DMA x_T window [c, 6, 130] per chunk straight into SBUF
  - h = W1^T x (PE, K=c contracted in 2 chunks of 128) -> PSUM
  - hrelu = Relu(scale*h + bias) on ACT (BN affine folded in); ones row via GPSIMD
  - kerT[n,144] = hrelu_aug^T @ w2aug (PE, bias via ones row) -> kernel output
  - kerb_k[128,n] = w2xk_k^T @ hrelu_aug (PE): group-broadcast kernels,
    w2xk_k[d, m] = w2aug[d, k*16 + m%16]
  - involution products P_jk = xT_shift_k * kerb_k on DVE (9 mults/chunk)
  - sum over k on PE: out_j += I^T @ P_jk (accumulating identity matmuls)
  - ACT copies sums PSUM->SBUF; DMA to channel-major out_T; host transposes back.
"""
import os
import sys
import numpy as np

sys.path.insert(0, "/opt/trn_rl_repo")

import concourse.bass as bass
import concourse.bacc as bacc
import concourse.tile as tile
from concourse import mybir
from concourse.bass_utils import run_bass_kernel_spmd
from concourse.masks import make_identity

BN_EPS = 1e-3
B, H, W, C = 4, 128, 128, 256
G, K, RED = 16, 3, 4
K2 = K * K
CR = C // RED          # 64
E = K2 * G             # 144
ROWS = 64              # output rows per core
SLAB = ROWS + 2        # input rows incl halo
R = 4                  # rows per block
NBLK = ROWS // R
N = R * W              # 512 pixels per block
WH = W + 2             # w incl halo cols
F32 = mybir.dt.float32

_CACHE = {}


def _build_program():
    nc = bacc.Bacc("TRN2", target_bir_lowering=False, debug=False, num_devices=8)

    x_dram = nc.dram_tensor("x_t", [2, 128, SLAB, WH], F32, kind="ExternalInput").ap()
    w1_dram = nc.dram_tensor("w1c", [2, 128, CR], F32, kind="ExternalInput").ap()
    scale_dram = nc.dram_tensor("actscale", [CR, 1], F32, kind="ExternalInput").ap()
    bias_dram = nc.dram_tensor("actbias", [CR, 1], F32, kind="ExternalInput").ap()
    w2aug_dram = nc.dram_tensor("w2aug", [CR + 1, E], F32, kind="ExternalInput").ap()
    w2xk_dram = nc.dram_tensor("w2xk", [K2, CR + 1, 128], F32, kind="ExternalInput").ap()
    out_dram = nc.dram_tensor("out_t", [2, 128, ROWS, W], F32, kind="ExternalOutput").ap()
    ker_dram = nc.dram_tensor("ker_slab", [ROWS, W, E], F32, kind="ExternalOutput").ap()

    with tile.TileContext(nc) as tc:
        _kernel_body(tc, x_dram, w1_dram, scale_dram, bias_dram, w2aug_dram,
                     w2xk_dram, out_dram, ker_dram)
    nc.compile()
    return nc


def _kernel_body(tc, x_dram, w1_dram, scale_dram, bias_dram, w2aug_dram,
                 w2xk_dram, out_dram, ker_dram):
    nc = tc.nc
    from contextlib import ExitStack
    ctx = ExitStack()
    with ctx:
        consts = ctx.enter_context(tc.tile_pool(name="consts", bufs=1))
        xt_p = ctx.enter_context(tc.tile_pool(name="xt", bufs=3))
        hr_p = ctx.enter_context(tc.tile_pool(name="hr", bufs=3))
        stage_p = ctx.enter_context(tc.tile_pool(name="stage", bufs=4))
        prod_p = ctx.enter_context(tc.tile_pool(name="prod", bufs=20))
        ps_hk = ctx.enter_context(tc.tile_pool(name="ps_hk", bufs=2, space="PSUM"))
        ps_kb = ctx.enter_context(tc.tile_pool(name="ps_kb", bufs=3, space="PSUM"))
        ps_sum = ctx.enter_context(tc.tile_pool(name="ps_sum", bufs=1, space="PSUM"))

        # ---- constants ----
        w1_sb = consts.tile([128, 2, CR], F32)
        nc.sync.dma_start(out=w1_sb, in_=w1_dram.rearrange("j p d -> p j d"))
        scale_sb = consts.tile([CR, 1], F32)
        nc.sync.dma_start(out=scale_sb, in_=scale_dram)
        bias_sb = consts.tile([CR, 1], F32)
        nc.sync.dma_start(out=bias_sb, in_=bias_dram)
        w2aug_sb = consts.tile([CR + 1, E], F32)
        nc.sync.dma_start(out=w2aug_sb, in_=w2aug_dram)
        w2xk_sb = consts.tile([CR + 1, K2, 128], F32)
        nc.sync.dma_start(out=w2xk_sb, in_=w2xk_dram.rearrange("k d m -> d k m"))
        ident = consts.tile([128, 128], F32)
        make_identity(nc, ident)

        for blk in range(NBLK):
            r0 = R * blk

            # ---- load channel-major x window [128, chunk, 6, 130] ----
            xT = xt_p.tile([128, 2, R + 2, WH], F32, tag="xT")
            for j in range(2):
                nc.sync.dma_start(out=xT[:, j], in_=x_dram[j, :, r0:r0 + R + 2, :])

            # ---- kernel generation ----
            h_ps = ps_hk.tile([CR, N], F32, tag="hk")
            for j in range(2):
                nc.tensor.matmul(h_ps, lhsT=w1_sb[:, j, :],
                                 rhs=xT[:, j, 1:R + 1, 1:W + 1],
                                 start=(j == 0), stop=(j == 1))
            hrelu_aug = hr_p.tile([CR + 1, N], F32, tag="hrelu")
            nc.gpsimd.memset(hrelu_aug[CR:CR + 1, :], 1.0)
            nc.scalar.activation(hrelu_aug[0:CR, :], h_ps,
                                 mybir.ActivationFunctionType.Relu,
                                 bias=bias_sb, scale=scale_sb)

            ksb = stage_p.tile([128, R, E], F32, tag="ksb")
            for half in range(2):
                kt = ps_hk.tile([128, 2, E], F32, tag="hk")
                for i2 in range(2):
                    i = half * 2 + i2
                    nc.tensor.matmul(kt[:, i2, :],
                                     lhsT=hrelu_aug[:, i * 128:(i + 1) * 128],
                                     rhs=w2aug_sb, start=True, stop=True)
                nc.scalar.copy(ksb[:, half * 2:half * 2 + 2, :], kt)
            nc.sync.dma_start(out=ker_dram[r0:r0 + R].rearrange("r w e -> w r e"),
                              in_=ksb)

            # ---- group-broadcast kernels + involution ----
            sums = []
            for j in range(2):
                sums.append(ps_sum.tile([128, R, W], F32, tag=f"sum{j}",
                                        name=f"sum{j}"))
            prods = []
            for k in range(K2):
                di, dj = k // 3, k % 3
                kb = ps_kb.tile([128, R, W], F32, tag="kerb")
                nc.tensor.matmul(kb, lhsT=w2xk_sb[:, k, :], rhs=hrelu_aug,
                                 start=True, stop=True)
                for j in range(2):
                    prod = prod_p.tile([128, R, W], F32, tag="prod",
                                       name=f"prod{k}_{j}")
                    nc.vector.tensor_mul(prod, xT[:, j, di:di + R, dj:dj + W], kb)
                    prods.append(prod)
            for k in range(K2):
                for j in range(2):
                    nc.tensor.matmul(sums[j], lhsT=ident, rhs=prods[2 * k + j],
                                     start=(k == 0), stop=(k == K2 - 1),
                                     skip_group_check=True)

            # ---- copy out and store (channel-major; host transposes back) ----
            for j in range(2):
                osb = stage_p.tile([128, R, W], F32, tag="osb")
                nc.scalar.copy(osb, sums[j])
                nc.sync.dma_start(out=out_dram[j, :, r0:r0 + R, :], in_=osb)


def _host_prep(w1, b1, gamma, beta, bn_mean, bn_var, w2, b2):
    s = (gamma / np.sqrt(bn_var + BN_EPS)).astype(np.float32)
    actscale = np.ascontiguousarray(s[:, None])
    actbias = np.ascontiguousarray((((b1 - bn_mean) * s) + beta)[:, None]).astype(np.float32)
    w1c = np.ascontiguousarray(w1.reshape(2, 128, CR)).astype(np.float32)
    w2aug = np.vstack([w2, b2[None]]).astype(np.float32)
    m = np.arange(128)
    w2xk = np.ascontiguousarray(
        np.stack([w2aug[:, k * G + (m % G)] for k in range(K2)])).astype(np.float32)
    return actscale, actbias, w1c, w2aug, w2xk


def kernel(x, w1, b1, gamma, beta, bn_mean, bn_var, w2, b2, _profile=None):
    x = np.asarray(x, np.float32)
    actscale, actbias, w1c, w2aug, w2xk = _host_prep(
        np.asarray(w1, np.float32), np.asarray(b1, np.float32),
        np.asarray(gamma, np.float32), np.asarray(beta, np.float32),
        np.asarray(bn_mean, np.float32), np.asarray(bn_var, np.float32),
        np.asarray(w2, np.float32), np.asarray(b2, np.float32))

    if "nc" not in _CACHE:
        _CACHE["nc"] = _build_program()
    nc = _CACHE["nc"]

    in_maps = []
    for i in range(8):
        bi, h0 = i // 2, (i % 2) * ROWS
        xs = np.zeros((SLAB, W, C), np.float32)
        xs[1:1 + ROWS] = x[bi, h0:h0 + ROWS]
        if h0 > 0:
            xs[0] = x[bi, h0 - 1]
        if h0 + ROWS < H:
            xs[1 + ROWS] = x[bi, h0 + ROWS]
        # channel-major with zero halo cols: [2, 128, SLAB, W+2]
        xt = np.zeros((2, 128, SLAB, WH), np.float32)
        xt[:, :, :, 1:W + 1] = (
            xs.transpose(2, 0, 1).reshape(2, 128, SLAB, W))
        in_maps.append({
            "x_t": xt, "w1c": w1c, "actscale": actscale, "actbias": actbias,
            "w2aug": w2aug, "w2xk": w2xk,
        })

    kwargs = dict(_profile) if _profile else {}
    res = run_bass_kernel_spmd(nc, in_maps, list(range(8)), **kwargs)
    if _profile is not None:
        _CACHE["last_result"] = res

    out = np.empty((B, H, W, C), np.float32)
    ker = np.empty((B, H, W, E), np.float32)
    for i in range(8):
        bi, h0 = i // 2, (i % 2) * ROWS
        # out_t [2, 128, ROWS, W] -> [ROWS, W, 2*128]
        ot = res.results[i]["out_t"].reshape(C, ROWS, W)
        out[bi, h0:h0 + ROWS] = ot.transpose(1, 2, 0)
        ker[bi, h0:h0 + ROWS] = res.results[i]["ker_slab"]
    return out, ker.reshape(B, H, W, K2, 1, G)
